# revision 1
# baseline (speedup 1.0000x reference)
"""Fused SwiGLU MLP (gate/up/down) Trainium2 Bass kernel.

Problem: y = down( silu(x @ Wg^T) * (x @ Wu^T) ) with
  x  [B=2, S=2048, H=4096]  f32
  Wg [I=11008, H]           f32   (gate proj, [out,in])
  Wu [I=11008, H]           f32
  Wd [H, I]                 f32

Strategy: data-parallel over tokens across the 8 NeuronCores.
Each core gets T = 4096/8 = 512 tokens and the full (replicated) weights,
computing the entire MLP for its token shard.  No collectives; the host
just concatenates the 8 token shards.  Per-core work: 138.6 GFLOP
(compute-bound: ~1.8 ms at the 78.6 TFLOP/s f32 PE roofline) vs ~532 MiB
of HBM traffic (~1.5 ms at ~360 GB/s), overlapped.

Device-side layout decisions (all transposes/tiling done on HOST in numpy
so every device DMA is a plain contiguous partition-major copy):
  x_host  [128, HS=32, T]          x^T tiled: [p, hs, t] = x[t, hs*128+p]
  wg_host [IC=22, 16, 128, 2, 512] Wg^T tiled (I padded 11008->11264)
  wu_host same
  wd_host [IC, 8, 128, 4, 512]     Wd^T tiled
  y out   [TT=4, 128, H]           y[tt*128+p, o]  (natural token-major)

Per-core kernel (per i-chunk ic of 512 padded-I columns):
  gate/up:  psum_g/u[it][128i, T] += Wg^T[h,i].T @ x^T[h,t]   (32 h-subtiles)
  mid:      hm[it] = silu(psum_g) * psum_u          (ACT + DVE)
  down:     psum_y[128t, 512o]    += hm[is][:,tt].T-as-lhsT @ Wd^T[i,o]
            y_sbuf[tt] += psum_y                    (DVE accumulate)
"""

import numpy as np

import concourse.bass as bass
import concourse.mybir as mybir
import concourse.tile as tile
from concourse import bacc
from concourse.bass_utils import run_bass_kernel_spmd

F32 = mybir.dt.float32
F32R = mybir.dt.float32r
P = 128
ICW = 512  # i-chunk width (4 subtiles of 128)
OCW = 512  # o-chunk width

# full-size problem constants
B, S, H, I = 2, 2048, 4096, 11008
NCORES = 8
T = (B * S) // NCORES  # 512 tokens per core
IPAD = 11264           # 22 * 512


def build_nc(T, H, IPAD, wg_bufs=6, wd_bufs=4, hm_bufs=5, sg_bufs=2, mm_dt=F32,
             use_silu=True):
    assert T % P == 0 and T <= 512
    assert H % 512 == 0 and (H // P) % 2 == 0
    assert IPAD % ICW == 0
    HS = H // P       # h subtiles (contraction for gate/up)
    IC = IPAD // ICW  # i chunks
    NO = H // OCW     # o chunks
    TT = T // P       # token tiles

    nc = bacc.Bacc("TRN2", target_bir_lowering=False, debug=False)
    x_d = nc.dram_tensor("x", [P, HS, T], mm_dt, kind="ExternalInput").ap()
    wg_d = nc.dram_tensor("wg", [IC, HS // 2, P, 2, ICW], mm_dt, kind="ExternalInput").ap()
    wu_d = nc.dram_tensor("wu", [IC, HS // 2, P, 2, ICW], mm_dt, kind="ExternalInput").ap()
    wd_d = nc.dram_tensor("wd", [IC, NO, P, ICW // P, OCW], mm_dt, kind="ExternalInput").ap()
    y_d = nc.dram_tensor("y", [TT, P, H], F32, kind="ExternalOutput").ap()

    with tile.TileContext(nc) as tc:
        with (
            tc.tile_pool(name="xp", bufs=1) as xp,
            tc.tile_pool(name="yp", bufs=1) as yp,
            tc.tile_pool(name="wgp", bufs=wg_bufs) as wgp,
            tc.tile_pool(name="wup", bufs=wg_bufs) as wup,
            tc.tile_pool(name="wdp", bufs=wd_bufs) as wdp,
            tc.tile_pool(name="hmp", bufs=hm_bufs) as hmp,
            tc.tile_pool(name="sgp", bufs=sg_bufs) as sgp,
            tc.tile_pool(name="ps", bufs=8, space="PSUM") as ps,
        ):
            # resident x^T (8 MiB) and y accumulator (8 MiB)
            xt = xp.tile([P, HS, T], mm_dt)
            nc.sync.dma_start(out=xt, in_=x_d)
            yt = []
            for tt in range(TT):
                ytile = yp.tile([P, H], F32, name=f"y{tt}", tag=f"y{tt}")
                nc.vector.memset(ytile, 0.0)
                yt.append(ytile)

            for ic in range(IC):
                # ---- gate/up projections, accumulated over all h ----
                psg = [ps.tile([P, T], F32, tag="ps", name=f"psg{k}") for k in range(4)]
                psu = [ps.tile([P, T], F32, tag="ps", name=f"psu{k}") for k in range(4)]
                for j in range(HS // 2):
                    gt = wgp.tile([P, 2, ICW], mm_dt, tag="wg")
                    nc.sync.dma_start(out=gt, in_=wg_d[ic, j])
                    ut = wup.tile([P, 2, ICW], mm_dt, tag="wu")
                    nc.sync.dma_start(out=ut, in_=wu_d[ic, j])
                    for h2 in range(2):
                        hs = 2 * j + h2
                        first, last = hs == 0, hs == HS - 1
                        for it in range(4):
                            nc.tensor.matmul(
                                psg[it],
                                gt[:, h2, it * P:(it + 1) * P],
                                xt[:, hs, :],
                                start=first, stop=last,
                            )
                        for it in range(4):
                            nc.tensor.matmul(
                                psu[it],
                                ut[:, h2, it * P:(it + 1) * P],
                                xt[:, hs, :],
                                start=first, stop=last,
                            )
                # ---- silu(gate) * up -> hm tiles [i128, T] ----
                hms = []
                for it in range(4):
                    sg = sgp.tile([P, T], F32, tag="sg")
                    if use_silu:
                        # native HW silu: one ACT op frees psg immediately
                        nc.scalar.activation(
                            sg, psg[it], mybir.ActivationFunctionType.Silu
                        )
                    else:
                        # CoreSim lacks Silu: sigmoid + extra DVE mul
                        nc.scalar.activation(
                            sg, psg[it], mybir.ActivationFunctionType.Sigmoid
                        )
                        nc.vector.tensor_mul(sg, sg, psg[it])
                    hm = hmp.tile([P, T], mm_dt, tag="hm")
                    nc.vector.tensor_mul(hm, sg, psu[it])
                    hms.append(hm)
                # ---- down projection for this i-chunk ----
                ISUB = ICW // P
                for osc in range(NO):
                    # wd for this (ic, osc) in two half tiles to keep SBUF slim
                    wdts = []
                    for half in range(2):
                        wdt = wdp.tile([P, ISUB // 2, OCW], mm_dt, tag="wd", name=f"wd{half}")
                        nc.sync.dma_start(
                            out=wdt,
                            in_=wd_d[ic, osc, :, half * (ISUB // 2):(half + 1) * (ISUB // 2), :],
                        )
                        wdts.append(wdt)
                    for tt in range(TT):
                        py = ps.tile([P, OCW], F32, tag="ps", name="py")
                        for isub in range(ISUB):
                            nc.tensor.matmul(
                                py,
                                hms[isub][:, tt * P:(tt + 1) * P],
                                wdts[isub // (ISUB // 2)][:, isub % (ISUB // 2), :],
                                start=(isub == 0), stop=(isub == ISUB - 1),
                            )
                        osl = slice(osc * OCW, (osc + 1) * OCW)
                        nc.vector.tensor_add(yt[tt][:, osl], yt[tt][:, osl], py)

            for tt in range(TT):
                nc.sync.dma_start(out=y_d[tt], in_=yt[tt])

    nc.compile()
    return nc


def prep_weights(Wg, Wu, Wd, IPAD):
    """Host-side re-tiling of the weights into the device DMA layouts."""
    Iin, Hh = Wg.shape
    HS = Hh // P
    IC = IPAD // ICW
    NO = Hh // OCW
    f32 = np.float32

    Wg_p = np.zeros((IPAD, Hh), f32)
    Wg_p[:Iin] = Wg
    Wu_p = np.zeros((IPAD, Hh), f32)
    Wu_p[:Iin] = Wu
    Wd_p = np.zeros((Hh, IPAD), f32)
    Wd_p[:, :Iin] = Wd

    # wg[ic, j, p, h2, ii] = Wg_p[ic*ICW + ii, (2j+h2)*128 + p]
    wg_host = np.ascontiguousarray(
        Wg_p.reshape(IC, ICW, HS // 2, 2, P).transpose(0, 2, 4, 3, 1)
    )
    wu_host = np.ascontiguousarray(
        Wu_p.reshape(IC, ICW, HS // 2, 2, P).transpose(0, 2, 4, 3, 1)
    )
    # wd[ic, osc, p, isub, oo] = Wd_p[osc*OCW + oo, ic*ICW + isub*128 + p]
    wd_host = np.ascontiguousarray(
        Wd_p.reshape(NO, OCW, IC, ICW // P, P).transpose(2, 0, 4, 3, 1)
    )
    return wg_host, wu_host, wd_host


def prep_x_shard(x2, c, T):
    """x2 [tokens, H] -> core c's [128, HS, T] tile layout."""
    Hh = x2.shape[1]
    xs = x2[c * T:(c + 1) * T]  # [T, H]
    return np.ascontiguousarray(xs.reshape(T, Hh // P, P).transpose(2, 1, 0))


def run_on_cores(nc, in_maps, **kwargs):
    return run_bass_kernel_spmd(nc, in_maps, core_ids=list(range(len(in_maps))), **kwargs)


_NC_CACHE = {}

# matmul dtype mode: "f32" (exact, 4 PE cycles/row) or "f32r" (tf32-like,
# 1 PE cycle/row, ~2e-4 rel err)
MM_MODE = "f32r"


def _get_nc(mode=None):
    mode = mode or MM_MODE
    key = (T, H, IPAD, mode)
    if key not in _NC_CACHE:
        _NC_CACHE[key] = build_nc(T, H, IPAD, mm_dt=(F32R if mode == "f32r" else F32))
    return _NC_CACHE[key]


def kernel(x, Wg, Wu, Wd, _trace=False, _trace_kwargs=None, _mode=None):
    x = np.asarray(x, np.float32)
    Wg = np.asarray(Wg, np.float32)
    Wu = np.asarray(Wu, np.float32)
    Wd = np.asarray(Wd, np.float32)

    nc = _get_nc(_mode)
    wg_host, wu_host, wd_host = prep_weights(Wg, Wu, Wd, IPAD)
    x2 = x.reshape(B * S, H)
    in_maps = [
        {
            "x": prep_x_shard(x2, c, T),
            "wg": wg_host,
            "wu": wu_host,
            "wd": wd_host,
        }
        for c in range(NCORES)
    ]
    kwargs = {}
    if _trace:
        kwargs["trace"] = True
        kwargs.update(_trace_kwargs or {})
    res = run_on_cores(nc, in_maps, **kwargs)
    shards = [res.results[c]["y"].reshape(T, H) for c in range(NCORES)]
    y = np.concatenate(shards, axis=0).reshape(B, S, H)
    if _trace:
        return y, res
    return y



# revision 4
# speedup vs baseline: 1.1735x; 1.1735x over previous
"""Fused SwiGLU MLP (gate/up/down) Trainium2 Bass kernel.

Problem: y = down( silu(x @ Wg^T) * (x @ Wu^T) ) with
  x  [B=2, S=2048, H=4096]  f32
  Wg [I=11008, H]           f32   (gate proj, [out,in])
  Wu [I=11008, H]           f32
  Wd [H, I]                 f32

Strategy: data-parallel over tokens across the 8 NeuronCores.
Each core gets T = 4096/8 = 512 tokens and the full (replicated) weights,
computing the entire MLP for its token shard.  No collectives; the host
just concatenates the 8 token shards.  Per-core work: 138.6 GFLOP.

Matmul operands are bf16 (PSUM accumulation stays f32): same PE speed as
f32r (1 row/cycle -> 78.6 TF/s) but weights DMA halves to ~272 MiB/core
(vs ~545 f32, which saturated HBM at ~84% and starved the PE) and
LDWEIGHTS gets the 2x fast-weight-load path (f32r measured 186 ns — it
barely hid under the 213 ns matmul).  End-to-end rel err ~2.5e-3.

Two-phase, hm-resident schedule per core (PE never waits on PSUM reuse):

Phase 1 (gate/up, 5504 MMs): for each pair of I-subtiles (43 groups of
2x128 gate + 2x128 up rows), accumulate over all 32 h-subtiles into 4
PSUM banks; silu (ACT, in-place in PSUM) + mul (DVE) drain each group to
a resident bf16 hm[s] = [128i, 512t] SBUF tile (86 tiles, 86 KiB/part).
Groups double-buffer through the 8 PSUM banks, so the next group's MMs
never wait on the previous group's ACT/DVE drain.

Phase 2 (down, 2752 MMs): for each 512-wide output chunk osc (8 of
them), py[tt] [128t, 512o] accumulates ALL 86 I-subtiles in PSUM
(4 banks per osc, double-buffered across osc) — no DVE y-accumulate at
all.  Each result bank is copied once to SBUF and DMA'd out per (osc,tt)
so the writeback tail is ~one tile.

Device-side layouts (all transposes/tiling done on HOST in numpy so
every device DMA is a plain contiguous partition-major copy):
  x_d  [128, 32, T]           x^T tiled: [p, hs, t] = x[t, hs*128+p]
  wg_d [86, 128, 32, 128]     [s, p, hs, i] = Wg[s*128+i, hs*128+p]
  wu_d same
  wd_d [8, 22, 128, 4, 512]   [osc, j, p, k, o] = Wd[osc*512+o, (4j+k)*128+p]
                              (I padded 11008->11264 with zero rows; the
                              pad subtiles are never matmul'd)
  y_d  [4, 128, H]            y[tt*128+p, o]  f32
"""

import numpy as np
import ml_dtypes

import concourse.bass as bass
import concourse.mybir as mybir
import concourse.tile as tile
from concourse import bacc
from concourse.bass_utils import run_bass_kernel_spmd

F32 = mybir.dt.float32
BF16 = mybir.dt.bfloat16
P = 128
OCW = 512   # output (o) chunk width for the down proj
GRP = 2     # gate/up I-subtiles per PSUM group (2 gate + 2 up = 4 banks)
QUAD = 4    # wd I-subtiles per DMA tile

# full-size problem constants
B, S, H, I = 2, 2048, 4096, 11008
NCORES = 8
T = (B * S) // NCORES  # 512 tokens per core


def build_nc(T, H, I, mm_dt=BF16, use_silu=True, w_bufs=8):
    HS = H // P            # h subtiles (contraction for gate/up)
    NS = I // P            # I subtiles
    NO = H // OCW          # output chunks for down proj
    TT = T // P            # token tiles
    NG = NS // GRP         # gate/up groups
    NQ = (NS + QUAD - 1) // QUAD  # wd DMA tiles per osc (last may be ragged)
    XC = 4                 # x DMA chunks
    assert T % P == 0 and T <= 512
    assert HS % XC == 0 and NS % GRP == 0

    nc = bacc.Bacc("TRN2", target_bir_lowering=False, debug=False)
    x_d = nc.dram_tensor("x", [P, HS, T], mm_dt, kind="ExternalInput").ap()
    wg_d = nc.dram_tensor("wg", [NS, P, HS, P], mm_dt, kind="ExternalInput").ap()
    wu_d = nc.dram_tensor("wu", [NS, P, HS, P], mm_dt, kind="ExternalInput").ap()
    wd_d = nc.dram_tensor("wd", [NO, NQ, P, QUAD, OCW], mm_dt, kind="ExternalInput").ap()
    y_d = nc.dram_tensor("y", [TT, P, H], F32, kind="ExternalOutput").ap()

    with tile.TileContext(nc) as tc:
        with (
            tc.tile_pool(name="xp", bufs=XC) as xp,
            tc.tile_pool(name="hmp", bufs=NS) as hmp,
            tc.tile_pool(name="wp", bufs=w_bufs) as wp,
            tc.tile_pool(name="sgp", bufs=2) as sgp,
            tc.tile_pool(name="yp", bufs=2) as yp,
            tc.tile_pool(name="ps", bufs=8, space="PSUM") as ps,
        ):
            # resident x^T, loaded in XC chunks so the first MMs start early
            xts = []
            hc = HS // XC
            for c in range(XC):
                xt = xp.tile([P, hc, T], mm_dt, name=f"x{c}", tag="x")
                nc.sync.dma_start(out=xt, in_=x_d[:, c * hc:(c + 1) * hc, :])
                xts.append(xt)

            def xs(hs):
                return xts[hs // hc][:, hs % hc, :]

            # ---- phase 1: gate/up -> hm[s] (resident bf16) ----
            hms = []
            for g in range(NG):
                subs = range(g * GRP, (g + 1) * GRP)
                gts, uts = [], []
                for s in subs:
                    gt = wp.tile([P, HS, P], mm_dt, tag="w", name=f"wg{s}")
                    nc.sync.dma_start(out=gt, in_=wg_d[s])
                    gts.append(gt)
                for s in subs:
                    ut = wp.tile([P, HS, P], mm_dt, tag="w", name=f"wu{s}")
                    nc.sync.dma_start(out=ut, in_=wu_d[s])
                    uts.append(ut)
                psg = [ps.tile([P, T], F32, tag="ps", name=f"psg{k}") for k in range(GRP)]
                psu = [ps.tile([P, T], F32, tag="ps", name=f"psu{k}") for k in range(GRP)]
                for hs in range(HS):
                    first, last = hs == 0, hs == HS - 1
                    for k in range(GRP):
                        nc.tensor.matmul(psg[k], gts[k][:, hs, :], xs(hs),
                                         start=first, stop=last)
                    for k in range(GRP):
                        nc.tensor.matmul(psu[k], uts[k][:, hs, :], xs(hs),
                                         start=first, stop=last)
                for k in range(GRP):
                    hm = hmp.tile([P, T], mm_dt, tag="hm", name=f"hm{g * GRP + k}")
                    if use_silu:
                        # native HW silu; DVE can read at most ONE PSUM
                        # operand, so silu lands in SBUF first
                        sg = sgp.tile([P, T], F32, tag="sg")
                        nc.scalar.activation(sg, psg[k],
                                             mybir.ActivationFunctionType.Silu)
                        nc.vector.tensor_mul(hm, sg, psu[k])
                    else:
                        # CoreSim lacks Silu: sigmoid (not in-place) + extra mul
                        sg = sgp.tile([P, T], F32, tag="sg")
                        nc.scalar.activation(sg, psg[k],
                                             mybir.ActivationFunctionType.Sigmoid)
                        nc.vector.tensor_mul(sg, sg, psg[k])
                        nc.vector.tensor_mul(hm, sg, psu[k])
                    hms.append(hm)

            # ---- phase 2: down proj, full-I accumulation in PSUM ----
            for osc in range(NO):
                wdts = []
                for j in range(NQ):
                    wdt = wp.tile([P, QUAD, OCW], mm_dt, tag="w", name=f"wd{osc}_{j}")
                    nc.sync.dma_start(out=wdt, in_=wd_d[osc, j])
                    wdts.append(wdt)
                pys = [ps.tile([P, OCW], F32, tag="ps", name=f"py{tt}")
                       for tt in range(TT)]
                for s in range(NS):
                    j, kq = divmod(s, QUAD)
                    first, last = s == 0, s == NS - 1
                    for tt in range(TT):
                        nc.tensor.matmul(pys[tt], hms[s][:, tt * P:(tt + 1) * P],
                                         wdts[j][:, kq, :], start=first, stop=last)
                osl = slice(osc * OCW, (osc + 1) * OCW)
                for tt in range(TT):
                    yt = yp.tile([P, OCW], F32, tag="y")
                    nc.vector.tensor_copy(yt, pys[tt])
                    nc.sync.dma_start(out=y_d[tt, :, osl], in_=yt)

    nc.compile()
    return nc


def _to_mm_np(a, mm_dt):
    if mm_dt == BF16:
        return a.astype(ml_dtypes.bfloat16)
    return np.ascontiguousarray(a, dtype=np.float32)


def prep_weights(Wg, Wu, Wd, mm_dt=BF16):
    """Host-side re-tiling of the weights into the device DMA layouts."""
    Iin, Hh = Wg.shape
    HS = Hh // P
    NS = Iin // P
    NO = Hh // OCW
    NQ = (NS + QUAD - 1) // QUAD
    NSP = NQ * QUAD

    # wg[s, p, hs, i] = Wg[s*128+i, hs*128+p]
    wg_host = Wg.reshape(NS, P, HS, P).transpose(0, 3, 2, 1)
    wu_host = Wu.reshape(NS, P, HS, P).transpose(0, 3, 2, 1)
    # wd[osc, j, p, k, o] = Wd_pad[osc*512+o, (4j+k)*128+p]
    Wd_pad = np.zeros((Hh, NSP * P), np.float32)
    Wd_pad[:, :Iin] = Wd
    wd_host = Wd_pad.reshape(NO, OCW, NQ, QUAD, P).transpose(0, 2, 4, 3, 1)
    return (_to_mm_np(np.ascontiguousarray(wg_host), mm_dt),
            _to_mm_np(np.ascontiguousarray(wu_host), mm_dt),
            _to_mm_np(np.ascontiguousarray(wd_host), mm_dt))


def prep_x_shard(x2, c, T, mm_dt=BF16):
    """x2 [tokens, H] -> core c's [128, HS, T] tile layout."""
    Hh = x2.shape[1]
    xs = x2[c * T:(c + 1) * T]  # [T, H]
    return _to_mm_np(
        np.ascontiguousarray(xs.reshape(T, Hh // P, P).transpose(2, 1, 0)), mm_dt)


def run_on_cores(nc, in_maps, **kwargs):
    return run_bass_kernel_spmd(nc, in_maps, core_ids=list(range(len(in_maps))), **kwargs)


_NC_CACHE = {}

# matmul dtype mode: "bf16" (1 PE cycle/row, FWL weight loads, ~2.5e-3 rel
# err) or "f32" (exact, 4 PE cycles/row, for CoreSim checks)
MM_MODE = "bf16"


def _get_nc(mode=None):
    mode = mode or MM_MODE
    key = (T, H, I, mode)
    if key not in _NC_CACHE:
        _NC_CACHE[key] = build_nc(T, H, I, mm_dt=(BF16 if mode == "bf16" else F32))
    return _NC_CACHE[key]


def kernel(x, Wg, Wu, Wd, _trace=False, _trace_kwargs=None, _mode=None):
    x = np.asarray(x, np.float32)
    Wg = np.asarray(Wg, np.float32)
    Wu = np.asarray(Wu, np.float32)
    Wd = np.asarray(Wd, np.float32)
    mode = _mode or MM_MODE
    mm_dt = BF16 if mode == "bf16" else F32

    nc = _get_nc(mode)
    wg_host, wu_host, wd_host = prep_weights(Wg, Wu, Wd, mm_dt)
    x2 = x.reshape(B * S, H)
    in_maps = [
        {
            "x": prep_x_shard(x2, c, T, mm_dt),
            "wg": wg_host,
            "wu": wu_host,
            "wd": wd_host,
        }
        for c in range(NCORES)
    ]
    kwargs = {}
    if _trace:
        kwargs["trace"] = True
        kwargs.update(_trace_kwargs or {})
    res = run_on_cores(nc, in_maps, **kwargs)
    shards = [res.results[c]["y"].reshape(T, H) for c in range(NCORES)]
    y = np.concatenate(shards, axis=0).reshape(B, S, H)
    if _trace:
        return y, res
    return y


# revision 8
# speedup vs baseline: 1.1742x; 1.0005x over previous
"""Fused SwiGLU MLP (gate/up/down) Trainium2 Bass kernel.

Problem: y = down( silu(x @ Wg^T) * (x @ Wu^T) ) with
  x  [B=2, S=2048, H=4096]  f32
  Wg [I=11008, H]           f32   (gate proj, [out,in])
  Wu [I=11008, H]           f32
  Wd [H, I]                 f32

Strategy: data-parallel over tokens across the 8 NeuronCores.
Each core gets T = 4096/8 = 512 tokens and the full (replicated) weights,
computing the entire MLP for its token shard.  No collectives; the host
just concatenates the 8 token shards.  Per-core work: 138.6 GFLOP.

Matmul operands are bf16 (PSUM accumulation stays f32): same PE speed as
f32r (1 row/cycle -> 78.6 TF/s) but weights DMA halves to ~272 MiB/core
(vs ~545 f32, which saturated HBM at ~84% and starved the PE) and
LDWEIGHTS gets the 2x fast-weight-load path (f32r measured 186 ns — it
barely hid under the 213 ns matmul).  End-to-end rel err ~2.5e-3.

Two-phase, hm-resident schedule per core (PE never waits on PSUM reuse):

Phase 1 (gate/up, 5504 MMs): for each pair of I-subtiles (43 groups of
2x128 gate + 2x128 up rows), accumulate over all 32 h-subtiles into 4
PSUM banks; silu (ACT, in-place in PSUM) + mul (DVE) drain each group to
a resident bf16 hm[s] = [128i, 512t] SBUF tile (86 tiles, 86 KiB/part).
Groups double-buffer through the 8 PSUM banks, so the next group's MMs
never wait on the previous group's ACT/DVE drain.

Phase 2 (down, 2752 MMs): for each 512-wide output chunk osc (8 of
them), py[tt] [128t, 512o] accumulates ALL 86 I-subtiles in PSUM
(4 banks per osc, double-buffered across osc) — no DVE y-accumulate at
all.  Each result bank is copied once to SBUF and DMA'd out per (osc,tt)
so the writeback tail is ~one tile.

Device-side layouts (all transposes/tiling done on HOST in numpy so
every device DMA is a plain contiguous partition-major copy):
  x_d  [128, 32, T]           x^T tiled: [p, hs, t] = x[t, hs*128+p]
  wg_d [86, 128, 32, 128]     [s, p, hs, i] = Wg[s*128+i, hs*128+p]
  wu_d same
  wd_d [8, 22, 128, 4, 512]   [osc, j, p, k, o] = Wd[osc*512+o, (4j+k)*128+p]
                              (I padded 11008->11264 with zero rows; the
                              pad subtiles are never matmul'd)
  y_d  [4, 128, H]            y[tt*128+p, o]  f32
"""

import numpy as np
import ml_dtypes

import concourse.bass as bass
import concourse.mybir as mybir
import concourse.tile as tile
from concourse import bacc
from concourse.bass_utils import run_bass_kernel_spmd

F32 = mybir.dt.float32
BF16 = mybir.dt.bfloat16
P = 128
OCW = 512   # output (o) chunk width for the down proj
GRP = 2     # gate/up I-subtiles per PSUM group (2 gate + 2 up = 4 banks)
QUAD = 4    # wd I-subtiles per DMA tile

# full-size problem constants
B, S, H, I = 2, 2048, 4096, 11008
NCORES = 8
T = (B * S) // NCORES  # 512 tokens per core


def build_nc(T, H, I, mm_dt=BF16, use_silu=True, w_bufs=8):
    HS = H // P            # h subtiles (contraction for gate/up)
    NS = I // P            # I subtiles
    NO = H // OCW          # output chunks for down proj
    TT = T // P            # token tiles
    NG = NS // GRP         # gate/up groups
    NQ = (NS + QUAD - 1) // QUAD  # wd DMA tiles per osc (last may be ragged)
    XC = 8 if HS % 8 == 0 else 4  # x DMA chunks
    WSL = 4 if HS % 4 == 0 else 1  # DMA slices per gate/up weight tile
    N_WARM = 20            # HAM warm-up matmuls during the startup DMA window
    assert T % P == 0 and T <= 512
    assert HS % XC == 0 and NS % GRP == 0

    nc = bacc.Bacc("TRN2", target_bir_lowering=False, debug=False)
    x_d = nc.dram_tensor("x", [P, HS, T], mm_dt, kind="ExternalInput").ap()
    wg_d = nc.dram_tensor("wg", [NS, P, HS, P], mm_dt, kind="ExternalInput").ap()
    wu_d = nc.dram_tensor("wu", [NS, P, HS, P], mm_dt, kind="ExternalInput").ap()
    wd_d = nc.dram_tensor("wd", [NO, NQ, P, QUAD, OCW], mm_dt, kind="ExternalInput").ap()
    y_d = nc.dram_tensor("y", [TT, P, H], F32, kind="ExternalOutput").ap()

    with tile.TileContext(nc) as tc:
        with (
            tc.tile_pool(name="xp", bufs=XC) as xp,
            tc.tile_pool(name="hmp", bufs=NS) as hmp,
            tc.tile_pool(name="wp", bufs=w_bufs) as wp,
            tc.tile_pool(name="sgp", bufs=2) as sgp,
            tc.tile_pool(name="yp", bufs=2) as yp,
            tc.tile_pool(name="ps", bufs=8, space="PSUM") as ps,
        ):
            # dummy zeroed operands for the PE warm-up matmuls
            dwt = xp.tile([P, P], mm_dt, name="dwt", tag="dw", bufs=1)
            dxt = xp.tile([P, T], mm_dt, name="dxt", tag="dx", bufs=1)
            nc.vector.memset(dwt, 0.0)
            nc.vector.memset(dxt, 0.0)

            # resident x^T, loaded in XC chunks so the first MMs start early
            xts = []
            hc = HS // XC
            for c in range(XC):
                xt = xp.tile([P, hc, T], mm_dt, name=f"x{c}", tag="x")
                nc.sync.dma_start(out=xt, in_=x_d[:, c * hc:(c + 1) * hc, :])
                xts.append(xt)

            def xs(hs):
                return xts[hs // hc][:, hs % hc, :]

            # ---- phase 1: gate/up -> hm[s] (resident bf16) ----
            hms = []
            for g in range(NG):
                subs = range(g * GRP, (g + 1) * GRP)
                hsl = HS // WSL
                gts, uts = [], []
                for s in subs:
                    gt = wp.tile([P, HS, P], mm_dt, tag="w", name=f"wg{s}")
                    for c in range(WSL):
                        sl = slice(c * hsl, (c + 1) * hsl)
                        nc.sync.dma_start(out=gt[:, sl, :], in_=wg_d[s, :, sl, :])
                    gts.append(gt)
                for s in subs:
                    ut = wp.tile([P, HS, P], mm_dt, tag="w", name=f"wu{s}")
                    for c in range(WSL):
                        sl = slice(c * hsl, (c + 1) * hsl)
                        nc.sync.dma_start(out=ut[:, sl, :], in_=wu_d[s, :, sl, :])
                    uts.append(ut)
                psg = [ps.tile([P, T], F32, tag="ps", name=f"psg{k}") for k in range(GRP)]
                psu = [ps.tile([P, T], F32, tag="ps", name=f"psu{k}") for k in range(GRP)]
                if g == 0:
                    # warm the PE clock (HAM) while the first DMAs land; the
                    # real hs=0 matmul below restarts the bank with start=True
                    for w in range(N_WARM):
                        nc.tensor.matmul(psg[0], dwt, dxt,
                                         start=(w == 0), stop=(w == N_WARM - 1))
                for hs in range(HS):
                    first, last = hs == 0, hs == HS - 1
                    for k in range(GRP):
                        nc.tensor.matmul(psg[k], gts[k][:, hs, :], xs(hs),
                                         start=first, stop=last)
                    for k in range(GRP):
                        nc.tensor.matmul(psu[k], uts[k][:, hs, :], xs(hs),
                                         start=first, stop=last)
                for k in range(GRP):
                    hm = hmp.tile([P, T], mm_dt, tag="hm", name=f"hm{g * GRP + k}")
                    if use_silu:
                        # native HW silu; DVE can read at most ONE PSUM
                        # operand, so silu lands in SBUF first
                        sg = sgp.tile([P, T], F32, tag="sg")
                        nc.scalar.activation(sg, psg[k],
                                             mybir.ActivationFunctionType.Silu)
                        nc.vector.tensor_mul(hm, sg, psu[k])
                    else:
                        # CoreSim lacks Silu: sigmoid (not in-place) + extra mul
                        sg = sgp.tile([P, T], F32, tag="sg")
                        nc.scalar.activation(sg, psg[k],
                                             mybir.ActivationFunctionType.Sigmoid)
                        nc.vector.tensor_mul(sg, sg, psg[k])
                        nc.vector.tensor_mul(hm, sg, psu[k])
                    hms.append(hm)

            # ---- phase 2: down proj, full-I accumulation in PSUM ----
            for osc in range(NO):
                wdts = []
                for j in range(NQ):
                    wdt = wp.tile([P, QUAD, OCW], mm_dt, tag="w", name=f"wd{osc}_{j}")
                    nc.sync.dma_start(out=wdt, in_=wd_d[osc, j])
                    wdts.append(wdt)
                pys = [ps.tile([P, OCW], F32, tag="ps", name=f"py{tt}")
                       for tt in range(TT)]
                for s in range(NS):
                    j, kq = divmod(s, QUAD)
                    first, last = s == 0, s == NS - 1
                    for tt in range(TT):
                        nc.tensor.matmul(pys[tt], hms[s][:, tt * P:(tt + 1) * P],
                                         wdts[j][:, kq, :], start=first, stop=last)
                osl = slice(osc * OCW, (osc + 1) * OCW)
                for tt in range(TT):
                    yt = yp.tile([P, OCW], F32, tag="y")
                    nc.vector.tensor_copy(yt, pys[tt])
                    nc.sync.dma_start(out=y_d[tt, :, osl], in_=yt)

    nc.compile()
    return nc


def _to_mm_np(a, mm_dt):
    if mm_dt == BF16:
        return a.astype(ml_dtypes.bfloat16)
    return np.ascontiguousarray(a, dtype=np.float32)


def prep_weights(Wg, Wu, Wd, mm_dt=BF16):
    """Host-side re-tiling of the weights into the device DMA layouts."""
    Iin, Hh = Wg.shape
    HS = Hh // P
    NS = Iin // P
    NO = Hh // OCW
    NQ = (NS + QUAD - 1) // QUAD
    NSP = NQ * QUAD

    # wg[s, p, hs, i] = Wg[s*128+i, hs*128+p]
    wg_host = Wg.reshape(NS, P, HS, P).transpose(0, 3, 2, 1)
    wu_host = Wu.reshape(NS, P, HS, P).transpose(0, 3, 2, 1)
    # wd[osc, j, p, k, o] = Wd_pad[osc*512+o, (4j+k)*128+p]
    Wd_pad = np.zeros((Hh, NSP * P), np.float32)
    Wd_pad[:, :Iin] = Wd
    wd_host = Wd_pad.reshape(NO, OCW, NQ, QUAD, P).transpose(0, 2, 4, 3, 1)
    return (_to_mm_np(np.ascontiguousarray(wg_host), mm_dt),
            _to_mm_np(np.ascontiguousarray(wu_host), mm_dt),
            _to_mm_np(np.ascontiguousarray(wd_host), mm_dt))


def prep_x_shard(x2, c, T, mm_dt=BF16):
    """x2 [tokens, H] -> core c's [128, HS, T] tile layout."""
    Hh = x2.shape[1]
    xs = x2[c * T:(c + 1) * T]  # [T, H]
    return _to_mm_np(
        np.ascontiguousarray(xs.reshape(T, Hh // P, P).transpose(2, 1, 0)), mm_dt)


def run_on_cores(nc, in_maps, **kwargs):
    return run_bass_kernel_spmd(nc, in_maps, core_ids=list(range(len(in_maps))), **kwargs)


_NC_CACHE = {}

# matmul dtype mode: "bf16" (1 PE cycle/row, FWL weight loads, ~2.5e-3 rel
# err) or "f32" (exact, 4 PE cycles/row, for CoreSim checks)
MM_MODE = "bf16"


def _get_nc(mode=None):
    mode = mode or MM_MODE
    key = (T, H, I, mode)
    if key not in _NC_CACHE:
        _NC_CACHE[key] = build_nc(T, H, I, mm_dt=(BF16 if mode == "bf16" else F32))
    return _NC_CACHE[key]


def kernel(x, Wg, Wu, Wd, _trace=False, _trace_kwargs=None, _mode=None):
    x = np.asarray(x, np.float32)
    Wg = np.asarray(Wg, np.float32)
    Wu = np.asarray(Wu, np.float32)
    Wd = np.asarray(Wd, np.float32)
    mode = _mode or MM_MODE
    mm_dt = BF16 if mode == "bf16" else F32

    nc = _get_nc(mode)
    wg_host, wu_host, wd_host = prep_weights(Wg, Wu, Wd, mm_dt)
    x2 = x.reshape(B * S, H)
    in_maps = [
        {
            "x": prep_x_shard(x2, c, T, mm_dt),
            "wg": wg_host,
            "wu": wu_host,
            "wd": wd_host,
        }
        for c in range(NCORES)
    ]
    kwargs = {}
    if _trace:
        kwargs["trace"] = True
        kwargs.update(_trace_kwargs or {})
    res = run_on_cores(nc, in_maps, **kwargs)
    shards = [res.results[c]["y"].reshape(T, H) for c in range(NCORES)]
    y = np.concatenate(shards, axis=0).reshape(B, S, H)
    if _trace:
        return y, res
    return y


# revision 11
# speedup vs baseline: 1.1781x; 1.0034x over previous
"""Fused SwiGLU MLP (gate/up/down) Trainium2 Bass kernel.

Problem: y = down( silu(x @ Wg^T) * (x @ Wu^T) ) with
  x  [B=2, S=2048, H=4096]  f32
  Wg [I=11008, H]           f32   (gate proj, [out,in])
  Wu [I=11008, H]           f32
  Wd [H, I]                 f32

Strategy: data-parallel over tokens across the 8 NeuronCores.
Each core gets T = 4096/8 = 512 tokens and the full (replicated) weights,
computing the entire MLP for its token shard.  No collectives; the host
just concatenates the 8 token shards.  Per-core work: 138.6 GFLOP.

Matmul operands are bf16 (PSUM accumulation stays f32): same PE speed as
f32r (1 row/cycle -> 78.6 TF/s) but weights DMA halves to ~272 MiB/core
(vs ~545 f32, which saturated HBM at ~84% and starved the PE) and
LDWEIGHTS gets the 2x fast-weight-load path (f32r measured 186 ns — it
barely hid under the 213 ns matmul).  End-to-end rel err ~2.5e-3.

Two-phase, hm-resident schedule per core (PE never waits on PSUM reuse):

Phase 1 (gate/up, 5504 MMs): for each pair of I-subtiles (43 groups of
2x128 gate + 2x128 up rows), accumulate over all 32 h-subtiles into 4
PSUM banks; silu (ACT, in-place in PSUM) + mul (DVE) drain each group to
a resident bf16 hm[s] = [128i, 512t] SBUF tile (86 tiles, 86 KiB/part).
Groups double-buffer through the 8 PSUM banks, so the next group's MMs
never wait on the previous group's ACT/DVE drain.

Phase 2 (down, 2752 MMs): for each 512-wide output chunk osc (8 of
them), py[tt] [128t, 512o] accumulates ALL 86 I-subtiles in PSUM
(4 banks per osc, double-buffered across osc) — no DVE y-accumulate at
all.  Each result bank is copied once to SBUF and DMA'd out per (osc,tt)
so the writeback tail is ~one tile.

Device-side layouts (all transposes/tiling done on HOST in numpy so
every device DMA is a plain contiguous partition-major copy):
  x_d  [128, 32, T]           x^T tiled: [p, hs, t] = x[t, hs*128+p]
  wg_d [86, 128, 32, 128]     [s, p, hs, i] = Wg[s*128+i, hs*128+p]
  wu_d same
  wd_d [8, 22, 128, 4, 512]   [osc, j, p, k, o] = Wd[osc*512+o, (4j+k)*128+p]
                              (I padded 11008->11264 with zero rows; the
                              pad subtiles are never matmul'd)
  y_d  [4, 128, H]            y[tt*128+p, o]  f32
"""

import numpy as np
import ml_dtypes

import concourse.bass as bass
import concourse.mybir as mybir
import concourse.tile as tile
from concourse import bacc
from concourse.bass_utils import run_bass_kernel_spmd

F32 = mybir.dt.float32
BF16 = mybir.dt.bfloat16
P = 128
OCW = 512   # output (o) chunk width for the down proj
GRP = 2     # gate/up I-subtiles per PSUM group (2 gate + 2 up = 4 banks)
QUAD = 4    # wd I-subtiles per DMA tile

# full-size problem constants
B, S, H, I = 2, 2048, 4096, 11008
NCORES = 8
T = (B * S) // NCORES  # 512 tokens per core


def build_nc(T, H, I, mm_dt=BF16, use_silu=True, w_bufs=8):
    HS = H // P            # h subtiles (contraction for gate/up)
    NS = I // P            # I subtiles
    NO = H // OCW          # output chunks for down proj
    TT = T // P            # token tiles
    NG = NS // GRP         # gate/up groups
    NQ = (NS + QUAD - 1) // QUAD  # wd DMA tiles per osc (last may be ragged)
    XC = 8 if HS % 8 == 0 else 4  # x DMA chunks
    WSL = 4 if HS % 4 == 0 else 1  # DMA slices per gate/up weight tile
    N_WARM = 12            # HAM warm-up matmuls during the startup DMA window
    assert T % P == 0 and T <= 512
    assert HS % XC == 0 and NS % GRP == 0

    nc = bacc.Bacc("TRN2", target_bir_lowering=False, debug=False)
    x_d = nc.dram_tensor("x", [P, HS, T], mm_dt, kind="ExternalInput").ap()
    wg_d = nc.dram_tensor("wg", [NS, P, HS, P], mm_dt, kind="ExternalInput").ap()
    wu_d = nc.dram_tensor("wu", [NS, P, HS, P], mm_dt, kind="ExternalInput").ap()
    wd_d = nc.dram_tensor("wd", [NO, NQ, P, QUAD, OCW], mm_dt, kind="ExternalInput").ap()
    y_d = nc.dram_tensor("y", [TT, P, H], F32, kind="ExternalOutput").ap()

    with tile.TileContext(nc) as tc:
        with (
            tc.tile_pool(name="xp", bufs=XC) as xp,
            tc.tile_pool(name="hmp", bufs=NS) as hmp,
            tc.tile_pool(name="wp", bufs=w_bufs) as wp,
            tc.tile_pool(name="sgp", bufs=2) as sgp,
            tc.tile_pool(name="yp", bufs=2) as yp,
            tc.tile_pool(name="ps", bufs=8, space="PSUM") as ps,
        ):
            # dummy zeroed operands for the PE warm-up matmuls
            dwt = xp.tile([P, P], mm_dt, name="dwt", tag="dw", bufs=1)
            dxt = xp.tile([P, T], mm_dt, name="dxt", tag="dx", bufs=1)
            nc.vector.memset(dwt, 0.0)
            nc.vector.memset(dxt, 0.0)

            # resident x^T in XC chunks; DMAs are emitted inside group 0
            # below, interleaved with its weight slices in consumption order,
            # so the first matmul gates on ~1.5 MiB instead of all of x
            hc = HS // XC
            xts = [xp.tile([P, hc, T], mm_dt, name=f"x{c}", tag="x")
                   for c in range(XC)]

            def xs(hs):
                return xts[hs // hc][:, hs % hc, :]

            # ---- phase 1: gate/up -> hm[s] (resident bf16) ----
            hms = []
            for g in range(NG):
                subs = list(range(g * GRP, (g + 1) * GRP))
                hsl = HS // WSL
                gts = [wp.tile([P, HS, P], mm_dt, tag="w", name=f"wg{s}")
                       for s in subs]
                uts = [wp.tile([P, HS, P], mm_dt, tag="w", name=f"wu{s}")
                       for s in subs]
                srcs = ([(gts[k], wg_d[subs[k]]) for k in range(GRP)]
                        + [(uts[k], wu_d[subs[k]]) for k in range(GRP)])
                for c in range(WSL):
                    sl = slice(c * hsl, (c + 1) * hsl)
                    for tl, src in srcs:
                        nc.sync.dma_start(out=tl[:, sl, :], in_=src[:, sl, :])
                    if g == 0:
                        # x chunks covering this hs range, right behind the
                        # weight slices that consume them
                        for xc in range(c * hsl // hc, (c + 1) * hsl // hc):
                            nc.sync.dma_start(
                                out=xts[xc], in_=x_d[:, xc * hc:(xc + 1) * hc, :])
                psg = [ps.tile([P, T], F32, tag="ps", name=f"psg{k}") for k in range(GRP)]
                psu = [ps.tile([P, T], F32, tag="ps", name=f"psu{k}") for k in range(GRP)]
                if g == 0:
                    # warm the PE clock (HAM) while the first DMAs land; the
                    # real hs=0 matmul below restarts the bank with start=True
                    for w in range(N_WARM):
                        nc.tensor.matmul(psg[0], dwt, dxt,
                                         start=(w == 0), stop=(w == N_WARM - 1))
                for hs in range(HS):
                    first, last = hs == 0, hs == HS - 1
                    for k in range(GRP):
                        nc.tensor.matmul(psg[k], gts[k][:, hs, :], xs(hs),
                                         start=first, stop=last)
                    for k in range(GRP):
                        nc.tensor.matmul(psu[k], uts[k][:, hs, :], xs(hs),
                                         start=first, stop=last)
                for k in range(GRP):
                    hm = hmp.tile([P, T], mm_dt, tag="hm", name=f"hm{g * GRP + k}")
                    if use_silu:
                        # native HW silu; DVE can read at most ONE PSUM
                        # operand, so silu lands in SBUF first
                        sg = sgp.tile([P, T], F32, tag="sg")
                        nc.scalar.activation(sg, psg[k],
                                             mybir.ActivationFunctionType.Silu)
                        nc.vector.tensor_mul(hm, sg, psu[k])
                    else:
                        # CoreSim lacks Silu: sigmoid (not in-place) + extra mul
                        sg = sgp.tile([P, T], F32, tag="sg")
                        nc.scalar.activation(sg, psg[k],
                                             mybir.ActivationFunctionType.Sigmoid)
                        nc.vector.tensor_mul(sg, sg, psg[k])
                        nc.vector.tensor_mul(hm, sg, psu[k])
                    hms.append(hm)

            # ---- phase 2: down proj, full-I accumulation in PSUM ----
            for osc in range(NO):
                wdts = []
                for j in range(NQ):
                    wdt = wp.tile([P, QUAD, OCW], mm_dt, tag="w", name=f"wd{osc}_{j}")
                    nc.sync.dma_start(out=wdt, in_=wd_d[osc, j])
                    wdts.append(wdt)
                pys = [ps.tile([P, OCW], F32, tag="ps", name=f"py{tt}")
                       for tt in range(TT)]
                for s in range(NS):
                    j, kq = divmod(s, QUAD)
                    first, last = s == 0, s == NS - 1
                    for tt in range(TT):
                        nc.tensor.matmul(pys[tt], hms[s][:, tt * P:(tt + 1) * P],
                                         wdts[j][:, kq, :], start=first, stop=last)
                osl = slice(osc * OCW, (osc + 1) * OCW)
                for tt in range(TT):
                    yt = yp.tile([P, OCW], F32, tag="y")
                    nc.vector.tensor_copy(yt, pys[tt])
                    nc.sync.dma_start(out=y_d[tt, :, osl], in_=yt)

    nc.compile()
    return nc


def _to_mm_np(a, mm_dt):
    if mm_dt == BF16:
        return a.astype(ml_dtypes.bfloat16)
    return np.ascontiguousarray(a, dtype=np.float32)


def prep_weights(Wg, Wu, Wd, mm_dt=BF16):
    """Host-side re-tiling of the weights into the device DMA layouts."""
    Iin, Hh = Wg.shape
    HS = Hh // P
    NS = Iin // P
    NO = Hh // OCW
    NQ = (NS + QUAD - 1) // QUAD
    NSP = NQ * QUAD

    # wg[s, p, hs, i] = Wg[s*128+i, hs*128+p]
    wg_host = Wg.reshape(NS, P, HS, P).transpose(0, 3, 2, 1)
    wu_host = Wu.reshape(NS, P, HS, P).transpose(0, 3, 2, 1)
    # wd[osc, j, p, k, o] = Wd_pad[osc*512+o, (4j+k)*128+p]
    Wd_pad = np.zeros((Hh, NSP * P), np.float32)
    Wd_pad[:, :Iin] = Wd
    wd_host = Wd_pad.reshape(NO, OCW, NQ, QUAD, P).transpose(0, 2, 4, 3, 1)
    return (_to_mm_np(np.ascontiguousarray(wg_host), mm_dt),
            _to_mm_np(np.ascontiguousarray(wu_host), mm_dt),
            _to_mm_np(np.ascontiguousarray(wd_host), mm_dt))


def prep_x_shard(x2, c, T, mm_dt=BF16):
    """x2 [tokens, H] -> core c's [128, HS, T] tile layout."""
    Hh = x2.shape[1]
    xs = x2[c * T:(c + 1) * T]  # [T, H]
    return _to_mm_np(
        np.ascontiguousarray(xs.reshape(T, Hh // P, P).transpose(2, 1, 0)), mm_dt)


def run_on_cores(nc, in_maps, **kwargs):
    return run_bass_kernel_spmd(nc, in_maps, core_ids=list(range(len(in_maps))), **kwargs)


_NC_CACHE = {}

# matmul dtype mode: "bf16" (1 PE cycle/row, FWL weight loads, ~2.5e-3 rel
# err) or "f32" (exact, 4 PE cycles/row, for CoreSim checks)
MM_MODE = "bf16"


def _get_nc(mode=None):
    mode = mode or MM_MODE
    key = (T, H, I, mode)
    if key not in _NC_CACHE:
        _NC_CACHE[key] = build_nc(T, H, I, mm_dt=(BF16 if mode == "bf16" else F32))
    return _NC_CACHE[key]


def kernel(x, Wg, Wu, Wd, _trace=False, _trace_kwargs=None, _mode=None):
    x = np.asarray(x, np.float32)
    Wg = np.asarray(Wg, np.float32)
    Wu = np.asarray(Wu, np.float32)
    Wd = np.asarray(Wd, np.float32)
    mode = _mode or MM_MODE
    mm_dt = BF16 if mode == "bf16" else F32

    nc = _get_nc(mode)
    wg_host, wu_host, wd_host = prep_weights(Wg, Wu, Wd, mm_dt)
    x2 = x.reshape(B * S, H)
    in_maps = [
        {
            "x": prep_x_shard(x2, c, T, mm_dt),
            "wg": wg_host,
            "wu": wu_host,
            "wd": wd_host,
        }
        for c in range(NCORES)
    ]
    kwargs = {}
    if _trace:
        kwargs["trace"] = True
        kwargs.update(_trace_kwargs or {})
    res = run_on_cores(nc, in_maps, **kwargs)
    shards = [res.results[c]["y"].reshape(T, H) for c in range(NCORES)]
    y = np.concatenate(shards, axis=0).reshape(B, S, H)
    if _trace:
        return y, res
    return y
